# revision 21
# baseline (speedup 1.0000x reference)
"""Trainium2 Bass kernel for the MIGCL GNN autoencoder (8 NeuronCores).

Strategy:
  - Nodes row-sharded across 8 cores (6250 rows/core, padded to 6656).
  - Edges partitioned by destination (host-side), so segment-sum is local.
  - Source features all-gathered as bf16 tables in DRAM; per-edge rows
    fetched with indirect DMA; weighted segment-sum done as PE matmuls
    against host-built one-hot*weight matrices (M tiles).
  - All activations kept feature-major ([hid, rows]) on chip so chained
    matmuls need no transposes; BN folded into weights on host.
"""

import heapq
import numpy as np
import ml_dtypes

import concourse.bass as bass
import concourse.mybir as mybir
import concourse.tile as tile
from concourse import bacc
from concourse.bass_utils import run_bass_kernel_spmd
from concourse.masks import make_identity

bfloat16 = ml_dtypes.bfloat16
F32 = mybir.dt.float32
BF16 = mybir.dt.bfloat16
I32 = mybir.dt.int32
I16 = mybir.dt.int16
AF = mybir.ActivationFunctionType
ALU = mybir.AluOpType

NCORES = 8
P = 128


def cdiv(a, b):
    return (a + b - 1) // b


class Cfg:
    def __init__(self, N, D_IN, H0, H1, GH, EMB, D0, D1, DOUT, CH=512, W=32,
                 CS=32, LOCAP=32768):
        assert N % NCORES == 0
        self.N = N
        self.R = N // NCORES              # real rows per core
        self.RPAD = cdiv(self.R, CH) * CH  # padded rows per core
        assert self.RPAD % P == 0
        self.NTAB = NCORES * self.RPAD    # gather-table rows
        self.LOCAP = LOCAP                # int16 index reach per stream
        assert self.NTAB <= 2 * LOCAP, "table too large for lo/hi split"
        self.D_IN = D_IN
        self.KIN = cdiv(D_IN, P)          # input K tiles
        self.H0, self.H1, self.GH, self.EMB = H0, H1, GH, EMB
        self.D0, self.D1, self.DOUT = D0, D1, DOUT
        assert H0 % P == 0 and H1 == P and GH % P == 0 and EMB == P
        assert D0 == P and D1 % P == 0
        self.CH = CH                      # row chunk (matmul N)
        self.NCHUNK = self.RPAD // CH
        self.RT = self.RPAD // P          # row tiles per core
        self.W = W                        # dst block width
        self.NBLK = self.RPAD // W        # blocks per core
        self.NREAL = cdiv(self.R, W)      # blocks containing real rows
        self.CS = CS                      # gather chunk (slots per DMA)


FULL = Cfg(N=50000, D_IN=3000, H0=512, H1=128, GH=256, EMB=128,
           D0=128, D1=512, DOUT=3000)


# --------------------------------------------------------------------------
# host-side data prep
# --------------------------------------------------------------------------

def _fold(W, g, b, gbias, eps=0.001):
    """Fold eval-mode BatchNorm into the linear layer. Returns (W', bias')."""
    s = (np.asarray(g, np.float32) / np.sqrt(np.float32(1.0 + eps)))
    Wp = (np.asarray(W, np.float32) * s[None, :])
    bp = np.asarray(b, np.float32) * s + np.asarray(gbias, np.float32)
    return Wp, bp


def _colmajor_bias(v, nt):
    """[nt*128] vector -> [128, nt] column-per-tile per-partition layout."""
    return np.ascontiguousarray(
        np.asarray(v, np.float32).reshape(nt, P).T)


def _pad2(a, r, c, dtype):
    out = np.zeros((r, c), dtype=dtype)
    out[:a.shape[0], :a.shape[1]] = a
    return out


def _wrap_idx16(lin):
    """Linear per-edge indices (i = slot*128 + lane) -> the [128, NI/16]
    int16 layout dma_gather expects (16-partition wrap, replicated x8)."""
    NI = len(lin)
    assert NI % 16 == 0
    arr = lin.astype(np.int16).reshape(NI // 16, 16).T    # [16, NI/16]
    return np.ascontiguousarray(np.tile(arr, (8, 1)))


def build_edge_stream(idx, w, cfg, perms):
    """Partition edges by destination core and build lo/hi gather streams
    (int16 dma_gather indices + one-hot*weight M tiles) with shared
    per-block tile budgets.

    Returns [stream_lo, stream_hi]; each stream is a dict with keys
    budgets, S, base, gidx (per-core [128, S*8] i16), M (per-core
    [128, S*W] bf16). stream_hi may have S == 0.
    """
    src = np.asarray(idx[0]).astype(np.int64)
    dst = np.asarray(idx[1]).astype(np.int64)
    w = np.asarray(w, np.float32)
    R, W_, RPAD = cfg.R, cfg.W, cfg.RPAD
    LOCAP = cfg.LOCAP
    HIBASE = max(0, cfg.NTAB - LOCAP)

    pos = np.empty(cfg.N, np.int64)
    for c in range(NCORES):
        pos[c * R:(c + 1) * R] = c * RPAD + perms[c]
    tab_src = pos[src]

    # per (core, block): split edges into lo/hi tile groups
    per_core = []                       # (core, block) -> (lo_list, hi_list)
    nlo = np.zeros((NCORES, cfg.NBLK), np.int64)
    nhi = np.zeros((NCORES, cfg.NBLK), np.int64)
    for c in range(NCORES):
        m = (dst // R) == c
        ld = perms[c][dst[m] % R]
        ts, wc = tab_src[m], w[m]
        blk = ld // W_
        order = np.argsort(blk, kind='stable')
        ld, ts, wc, blk = ld[order], ts[order], wc[order], blk[order]
        starts = np.searchsorted(blk, np.arange(cfg.NBLK))
        ends = np.searchsorted(blk, np.arange(cfg.NBLK), side='right')
        blocks = []
        for b in range(cfg.NBLK):
            sl = slice(starts[b], ends[b])
            r, lw, lld = ts[sl], wc[sl], ld[sl]
            must_lo = r < HIBASE
            must_hi = r >= LOCAP
            flex = ~must_lo & ~must_hi
            ml, mh, fl = must_lo.sum(), must_hi.sum(), flex.sum()
            # pick x (flex edges sent to lo) minimizing total tiles
            xs = np.arange(fl + 1)
            cost = -(-(ml + xs) // P) + -(-(mh + fl - xs) // P)
            x = int(xs[np.argmin(cost)])
            fi = np.where(flex)[0]
            lo_sel = np.concatenate([np.where(must_lo)[0], fi[:x]])
            hi_sel = np.concatenate([np.where(must_hi)[0], fi[x:]])
            blocks.append(((r[lo_sel], lw[lo_sel], lld[lo_sel]),
                           (r[hi_sel] - HIBASE, lw[hi_sel], lld[hi_sel])))
            nlo[c, b] = len(lo_sel)
            nhi[c, b] = len(hi_sel)
        per_core.append(blocks)

    bud_lo = (-(-nlo // P)).max(axis=0)
    bud_hi = (-(-nhi // P)).max(axis=0)
    # every real block needs >= 1 tile total so the bias/copy runs
    empty = (bud_lo + bud_hi) == 0
    force = empty & (np.arange(cfg.NBLK) < cfg.NREAL)
    bud_lo[force] = 1

    streams = []
    for bud, base, side in ((bud_lo, 0, 0), (bud_hi, HIBASE, 1)):
        S = int(bud.sum())
        offs = np.concatenate([[0], np.cumsum(bud)]).astype(int)
        gidx_all, M_all = [], []
        for c in range(NCORES):
            lin = np.zeros(S * P, np.int64)
            M = np.zeros((P, S * W_), np.float32)
            for b in range(cfg.NBLK):
                if bud[b] == 0:
                    continue
                r, lw, lld = per_core[c][b][side]
                n = len(r)
                assert n <= bud[b] * P
                s0 = offs[b]
                ii = s0 * P + np.arange(n)          # linear positions
                lin[ii] = r
                slot = ii // P
                lane = ii % P
                M[lane, slot * W_ + (lld % W_)] = lw
            gidx_all.append(_wrap_idx16(lin))
            M_all.append(np.ascontiguousarray(M.astype(bfloat16)))
        streams.append(dict(budgets=bud, S=S, base=base, gidx=gidx_all,
                            M=M_all))
    return streams


def balance_perm(inputs, cfg, c):
    """Permute a core's local nodes so per-block (W dsts) edge counts are
    balanced. Greedy: highest-degree dsts first into least-loaded block."""
    R, W_ = cfg.R, cfg.W
    deg = np.zeros(R, np.int64)
    for key in ('fidx', 'sidx'):
        dst = np.asarray(inputs[key][1]).astype(np.int64)
        d = dst[(dst // R) == c] % R
        deg += np.bincount(d, minlength=R)
    nblk = cfg.NBLK
    order = np.argsort(-deg, kind='stable')
    slots = np.full(nblk, W_, np.int64)
    npad = cfg.RPAD - R
    bi = nblk - 1
    while npad > 0:
        take = min(npad, W_)
        slots[bi] -= take
        npad -= take
        bi -= 1
    perm = np.empty(R, np.int64)
    heap = [(0, b) for b in range(nblk) if slots[b] > 0]
    heapq.heapify(heap)
    fill = np.zeros(nblk, np.int64)
    for i in order:
        while True:
            l, b = heapq.heappop(heap)
            if fill[b] < slots[b]:
                break
        perm[i] = b * W_ + fill[b]
        fill[b] += 1
        if fill[b] < slots[b]:
            heapq.heappush(heap, (l + int(deg[i]), b))
    return perm


def prep_host(inputs, cfg, balance=True):
    """All host-side preprocessing. Returns (in_maps, meta)."""
    feat = np.asarray(inputs['feat'], np.float32)
    eW0p, eb0p = _fold(inputs['eW0'], inputs['eg0'], inputs['eb0'],
                       inputs['ebt0'])
    eW1p, eb1p = _fold(inputs['eW1'], inputs['eg1'], inputs['eb1'],
                       inputs['ebt1'])
    dW0p, db0p = _fold(inputs['dW0'], inputs['dg0'], inputs['db0'],
                       inputs['dbt0'])
    dW1p, db1p = _fold(inputs['dW1'], inputs['dg1'], inputs['db1'],
                       inputs['dbt1'])

    perms = []
    for c in range(NCORES):
        if balance:
            perms.append(balance_perm(inputs, cfg, c))
        else:
            perms.append(np.arange(cfg.R, dtype=np.int64))

    str_f = build_edge_stream(inputs['fidx'], inputs['fw'], cfg, perms)
    str_s = build_edge_stream(inputs['sidx'], inputs['sw'], cfg, perms)

    KP = cfg.KIN * P
    common = {
        'eW0p': _pad2(eW0p.astype(bfloat16), KP, cfg.H0, bfloat16),
        'ebt0': _colmajor_bias(eb0p, cfg.H0 // P),
        'ebt0n': _colmajor_bias(-eb0p, cfg.H0 // P),
        'eW1p': np.ascontiguousarray(eW1p.astype(bfloat16)),
        'ebt1': _colmajor_bias(eb1p, 1),
        'ebt1n': _colmajor_bias(-eb1p, 1),
        'gW1': np.ascontiguousarray(np.asarray(inputs['gW1'], np.float32)
                                    .astype(bfloat16)),
        'gb1': _colmajor_bias(np.asarray(inputs['gb1'], np.float32),
                              cfg.GH // P),
        'gW2': np.ascontiguousarray(np.asarray(inputs['gW2'], np.float32)
                                    .astype(bfloat16)),
        'gb2': _colmajor_bias(np.asarray(inputs['gb2'], np.float32), 1),
        'aWb': np.ascontiguousarray(
            np.tile(np.asarray(inputs['aW'], np.float32).reshape(1, cfg.EMB),
                    (P, 1))),
        'dW0p': np.ascontiguousarray(dW0p.astype(bfloat16)),
        'dbt0': _colmajor_bias(db0p, cfg.D1 // P),
        'dbt0n': _colmajor_bias(-db0p, cfg.D1 // P),
        'dW1p': np.ascontiguousarray(dW1p.astype(bfloat16)),
        'dbt1b': np.ascontiguousarray(
            np.tile(db1p.reshape(1, cfg.DOUT), (P, 1)).astype(np.float32)),
    }

    common['id32'] = np.eye(P, dtype=np.float32)
    common['id16'] = np.eye(P, dtype=bfloat16)

    in_maps = []
    for c in range(NCORES):
        fc = feat[c * cfg.R:(c + 1) * cfg.R]       # [R, D_IN]
        fT = np.zeros((KP, cfg.RPAD), np.float32)
        fT[:cfg.D_IN, perms[c]] = fc.T
        m = dict(common)
        m['featT'] = fT.astype(bfloat16)
        for g, streams in (('f', str_f), ('s', str_s)):
            for si, stm in enumerate(streams):
                if stm['S'] == 0:
                    continue
                m[f'gidx{g}{si}'] = stm['gidx'][c]
                m[f'M{g}{si}'] = stm['M'][c]
        in_maps.append(m)

    meta = {'str_f': str_f, 'str_s': str_s, 'perms': perms}
    return in_maps, meta


# --------------------------------------------------------------------------
# device program
# --------------------------------------------------------------------------

def _elu(nc, pool, psum, bias, nbias, out_slice, tag):
    """out = elu(psum + bias) = relu(u) + exp(min(u, 0)) - 1, cast to out."""
    sh = [psum.shape[0], psum.free_size()]
    r = pool.tile(sh, F32, name=f"elu_r_{tag}", tag="elu_r", bufs=2)
    nc.scalar.activation(r[:], psum, AF.Relu, bias=bias)
    nx = pool.tile(sh, F32, name=f"elu_n_{tag}", tag="elu_n", bufs=2)
    nc.scalar.activation(nx[:], psum, AF.Relu, bias=nbias, scale=-1.0)
    e = pool.tile(sh, F32, name=f"elu_e_{tag}", tag="elu_e", bufs=2)
    nc.scalar.activation(e[:], nx[:], AF.Exp, scale=-1.0)
    s = pool.tile(sh, F32, name=f"elu_s_{tag}", tag="elu_s", bufs=2)
    nc.vector.tensor_tensor(s[:], e[:], r[:], op=ALU.add)
    nc.vector.tensor_scalar(out_slice, s[:], 1.0, None, op0=ALU.subtract)


def build_program(cfg, str_f, str_s, debug_taps=False):
    nc = bacc.Bacc("TRN2", target_bir_lowering=False, debug=False,
                   num_devices=NCORES)
    KP = cfg.KIN * P

    # ---- I/O ----
    featT = nc.dram_tensor("featT", [KP, cfg.RPAD], BF16, kind="ExternalInput")
    # per-graph lo/hi gather streams: (gidx_dram, M_dram, budgets, S, base)
    gstreams = {}
    for g, streams in (('f', str_f), ('s', str_s)):
        lst = []
        for si, stm in enumerate(streams):
            if stm['S'] == 0:
                continue
            S = stm['S']
            gi = nc.dram_tensor(f"gidx{g}{si}", [P, S * 8], I16,
                                kind="ExternalInput")
            Md = nc.dram_tensor(f"M{g}{si}", [P, S * cfg.W], BF16,
                                kind="ExternalInput")
            lst.append((gi, Md, stm['budgets'], S, stm['base']))
        gstreams[g] = lst
    id32_d = nc.dram_tensor("id32", [P, P], F32, kind="ExternalInput")
    id16_d = nc.dram_tensor("id16", [P, P], BF16, kind="ExternalInput")
    eW0p = nc.dram_tensor("eW0p", [KP, cfg.H0], BF16, kind="ExternalInput")
    ebt0 = nc.dram_tensor("ebt0", [P, cfg.H0 // P], F32, kind="ExternalInput")
    ebt0n = nc.dram_tensor("ebt0n", [P, cfg.H0 // P], F32,
                           kind="ExternalInput")
    eW1p = nc.dram_tensor("eW1p", [cfg.H0, cfg.H1], BF16,
                          kind="ExternalInput")
    ebt1 = nc.dram_tensor("ebt1", [P, 1], F32, kind="ExternalInput")
    ebt1n = nc.dram_tensor("ebt1n", [P, 1], F32, kind="ExternalInput")
    gW1 = nc.dram_tensor("gW1", [cfg.H1, cfg.GH], BF16, kind="ExternalInput")
    gb1 = nc.dram_tensor("gb1", [P, cfg.GH // P], F32, kind="ExternalInput")
    gW2 = nc.dram_tensor("gW2", [cfg.GH, cfg.EMB], BF16, kind="ExternalInput")
    gb2 = nc.dram_tensor("gb2", [P, 1], F32, kind="ExternalInput")
    aWb = nc.dram_tensor("aWb", [P, cfg.EMB], F32, kind="ExternalInput")
    dW0p = nc.dram_tensor("dW0p", [cfg.EMB, cfg.D1], BF16,
                          kind="ExternalInput")
    dbt0 = nc.dram_tensor("dbt0", [P, cfg.D1 // P], F32, kind="ExternalInput")
    dbt0n = nc.dram_tensor("dbt0n", [P, cfg.D1 // P], F32,
                           kind="ExternalInput")
    dW1p = nc.dram_tensor("dW1p", [cfg.D1, cfg.DOUT], BF16,
                          kind="ExternalInput")
    dbt1b = nc.dram_tensor("dbt1b", [P, cfg.DOUT], F32, kind="ExternalInput")

    femb_o = nc.dram_tensor("femb_o", [cfg.RPAD, cfg.EMB], F32,
                            kind="ExternalOutput")
    semb_o = nc.dram_tensor("semb_o", [cfg.RPAD, cfg.EMB], F32,
                            kind="ExternalOutput")
    emb_o = nc.dram_tensor("emb_o", [cfg.RPAD, cfg.EMB], F32,
                           kind="ExternalOutput")
    de_o = nc.dram_tensor("de_o", [cfg.RPAD, cfg.DOUT], F32,
                          kind="ExternalOutput")
    if debug_taps:
        zf_dbg = nc.dram_tensor("zf_dbg", [cfg.NTAB, cfg.EMB], BF16,
                                kind="ExternalOutput")
        xf_dbg = nc.dram_tensor("xf_dbg", [P, cfg.RPAD], BF16,
                                kind="ExternalOutput")
        fem_dbg = nc.dram_tensor("fem_dbg", [P, cfg.RPAD], F32,
                                 kind="ExternalOutput")
        s2f_dbg = nc.dram_tensor("s2f_dbg", [cfg.NTAB, cfg.EMB], BF16,
                                 kind="ExternalOutput")

    rg = [list(range(NCORES))]
    MH = cfg.H0 // P
    MG = cfg.GH // P
    MD = cfg.D1 // P

    with tile.TileContext(nc) as tc:
        with (
            tc.tile_pool(name="wc", bufs=1) as wc,
            tc.tile_pool(name="ptr", bufs=1, space="PSUM") as ptr,
            tc.tile_pool(name="big", bufs=1) as bigp,
            tc.tile_pool(name="rowp", bufs=1) as rowp,
            tc.tile_pool(name="dram", bufs=1, space="DRAM") as dp,
        ):
            # ---- common weights / constants ----
            def wtile(name, src, shape, dtype=BF16):
                t = wc.tile(shape, dtype, name=name)
                nc.sync.dma_start(t[:], src)
                return t

            id32 = wtile("id32_sb", id32_d[:, :], [P, P], F32)
            id16 = wtile("id16_sb", id16_d[:, :], [P, P], BF16)

            eW1_sb = wc.tile([P, cfg.H0], BF16, name="eW1_sb")
            for k in range(MH):
                nc.sync.dma_start(eW1_sb[:, k * P:(k + 1) * P],
                                  eW1p[k * P:(k + 1) * P, :])
            gW1_sb = wtile("gW1_sb", gW1[:, :], [P, cfg.GH])
            gW2_sb = wc.tile([P, cfg.GH], BF16, name="gW2_sb")
            for k in range(MG):
                nc.sync.dma_start(gW2_sb[:, k * P:(k + 1) * P],
                                  gW2[k * P:(k + 1) * P, :])
            dW0_sb = wtile("dW0_sb", dW0p[:, :], [P, cfg.D1])
            ebt1_sb = wtile("ebt1_sb", ebt1[:, :], [P, 1], F32)
            ebt1n_sb = wtile("ebt1n_sb", ebt1n[:, :], [P, 1], F32)
            gb1_sb = wtile("gb1_sb", gb1[:, :], [P, cfg.GH // P], F32)
            gb2_sb = wtile("gb2_sb", gb2[:, :], [P, 1], F32)
            aWb_sb = wtile("aWb_sb", aWb[:, :], [P, cfg.EMB], F32)
            dbt0_sb = wtile("dbt0_sb", dbt0[:, :], [P, cfg.D1 // P], F32)
            dbt0n_sb = wtile("dbt0n_sb", dbt0n[:, :], [P, cfg.D1 // P], F32)
            ebt0_sb = wtile("ebt0_sb", ebt0[:, :], [P, cfg.H0 // P], F32)
            ebt0n_sb = wtile("ebt0n_sb", ebt0n[:, :], [P, cfg.H0 // P], F32)

            # ---- DRAM comm buffers ----
            z_loc = dp.tile([cfg.RPAD, cfg.EMB], BF16, name="z_loc")
            z_full = dp.tile([cfg.NTAB, cfg.EMB], BF16, name="z_full",
                             addr_space="Shared")
            s2f_loc = dp.tile([cfg.RPAD, cfg.EMB], BF16, name="s2f_loc")
            s2f_full = dp.tile([cfg.NTAB, cfg.EMB], BF16, name="s2f_full",
                               addr_space="Shared")
            s2s_loc = dp.tile([cfg.RPAD, cfg.EMB], BF16, name="s2s_loc")
            s2s_full = dp.tile([cfg.NTAB, cfg.EMB], BF16, name="s2s_full",
                               addr_space="Shared")

            BIGT = dict(tag="bigbf", bufs=5)

            def store_rows(srcT, dst_dram, dtype=BF16):
                """Transpose [feat, rows] -> row-major and DMA to DRAM."""
                for rt in range(cfg.RT):
                    rsl = slice(rt * P, (rt + 1) * P)
                    tp = ptr.tile([P, P], dtype,
                                  name=f"tp_{dst_dram.name}_{rt}",
                                  tag="tr", bufs=2)
                    ident = id16 if dtype == BF16 else id32
                    nc.tensor.transpose(tp[:], srcT[:, rsl], ident[:])
                    row = rowp.tile([P, P], dtype,
                                    name=f"row_{dst_dram.name}_{rt}",
                                    tag="rowst", bufs=3)
                    nc.scalar.copy(row[:], tp[:])
                    nc.sync.dma_start(dst_dram[rsl, :], row[:])

            # ================= encoder =================
            zT = bigp.tile([P, cfg.RPAD], BF16, name="zT", **BIGT)
            with (
                tc.tile_pool(name="encw", bufs=1) as ew,
                tc.tile_pool(name="encs", bufs=1) as st,
                tc.tile_pool(name="encp", bufs=1, space="PSUM") as epp,
            ):
                eW0_sb = []
                for k in range(cfg.KIN):
                    t = ew.tile([P, cfg.H0], BF16, name=f"eW0_{k}",
                                tag=f"ew0_{k}")
                    nc.sync.dma_start(t[:], eW0p[k * P:(k + 1) * P, :])
                    eW0_sb.append(t)
                for c in range(cfg.NCHUNK):
                    csl = slice(c * cfg.CH, (c + 1) * cfg.CH)
                    z1ps = [epp.tile([P, cfg.CH], F32, name=f"z1ps_{c}_{m}",
                                     tag=f"mmz1_{m}", bufs=1)
                            for m in range(MH)]
                    for k in range(cfg.KIN):
                        ft = st.tile([P, cfg.CH], BF16, name=f"ft_{c}_{k}",
                                     tag="ft", bufs=4)
                        nc.sync.dma_start(ft[:],
                                          featT[k * P:(k + 1) * P, csl])
                        for m in range(MH):
                            nc.tensor.matmul(
                                z1ps[m][:],
                                eW0_sb[k][:, m * P:(m + 1) * P], ft[:],
                                start=(k == 0), stop=(k == cfg.KIN - 1))
                    z1e = [st.tile([P, cfg.CH], BF16, name=f"z1e_{c}_{m}",
                                   tag=f"z1e_{m}", bufs=2)
                           for m in range(MH)]
                    for m in range(MH):
                        _elu(nc, st, z1ps[m][:], ebt0_sb[:, m:m + 1],
                             ebt0n_sb[:, m:m + 1], z1e[m][:], f"e_{c}_{m}")
                    z2ps = epp.tile([P, cfg.CH], F32, name=f"z2ps_{c}",
                                    tag="mm2", bufs=2)
                    for m in range(MH):
                        nc.tensor.matmul(z2ps[:],
                                         eW1_sb[:, m * P:(m + 1) * P],
                                         z1e[m][:],
                                         start=(m == 0), stop=(m == MH - 1))
                    _elu(nc, st, z2ps[:], ebt1_sb[:, :1], ebt1n_sb[:, :1],
                         zT[:, csl], f"z_{c}")

            store_rows(zT, z_loc)
            nc.gpsimd.collective_compute(
                "AllGather", ALU.bypass, replica_groups=rg,
                ins=[z_loc.opt()], outs=[z_full.opt()])
            if debug_taps:
                nc.sync.dma_start(zf_dbg[:, :], z_full[:, :])

            # ================= GCN phase =================
            with (
                tc.tile_pool(name="aggs", bufs=1) as st,
                tc.tile_pool(name="aggp", bufs=1, space="PSUM") as app,
                tc.tile_pool(name="bigf", bufs=1) as bigf,
            ):
                def aggregate(table, streams, outT, bias_col, out_f32,
                              label):
                    # chunked dma_gather per stream, then per-block matmuls
                    chunk_data = {}
                    offs = {}
                    for si, (gi_d, M_d, bud, S, base) in enumerate(streams):
                        offs[si] = np.concatenate(
                            [[0], np.cumsum(bud)]).astype(int)
                        tab_ap = (table[base:base + cfg.LOCAP, :]
                                  if base or cfg.NTAB > cfg.LOCAP
                                  else table[:, :])
                        sbufs = 3 if si == 0 else 2
                        for ci in range(cdiv(S, cfg.CS)):
                            s0 = ci * cfg.CS
                            s1 = min(S, s0 + cfg.CS)
                            ww = s1 - s0
                            gx = st.tile([P, ww * 8], I16,
                                         name=f"gx_{label}_{si}_{ci}",
                                         tag=f"gidx{si}", bufs=sbufs)
                            nc.sync.dma_start(gx[:],
                                              gi_d[:, s0 * 8:s1 * 8])
                            mb = st.tile([P, ww * cfg.W], BF16,
                                         name=f"mb_{label}_{si}_{ci}",
                                         tag=f"mbuf{si}", bufs=sbufs)
                            nc.sync.dma_start(mb[:], M_d[:, s0 * cfg.W:
                                                         s1 * cfg.W])
                            gb = st.tile([P, ww * P], BF16,
                                         name=f"gb_{label}_{si}_{ci}",
                                         tag=f"gbuf{si}", bufs=sbufs)
                            nc.gpsimd.dma_gather(
                                out_ap=gb[:].rearrange("p (s f) -> p s f",
                                                       f=P),
                                in_ap=tab_ap, idxs_ap=gx[:],
                                num_idxs=ww * P, num_idxs_reg=ww * P,
                                elem_size=P, single_packet=False)
                            chunk_data[(si, ci)] = (gb, mb)
                    for b in range(cfg.NBLK):
                        nts = [int(stm[2][b]) for stm in streams]
                        nb = sum(nts)
                        if nb == 0:
                            continue
                        ps = app.tile([P, cfg.W], F32,
                                      name=f"agg_{label}_{b}", tag="agg",
                                      bufs=3)
                        j = 0
                        for si, nt in enumerate(nts):
                            for t in range(nt):
                                s = offs[si][b] + t
                                ci, co = divmod(s, cfg.CS)
                                gb, mb = chunk_data[(si, ci)]
                                nc.tensor.matmul(
                                    ps[:], gb[:, co * P:(co + 1) * P],
                                    mb[:, co * cfg.W:(co + 1) * cfg.W],
                                    start=(j == 0), stop=(j == nb - 1))
                                j += 1
                        bsl = slice(b * cfg.W, (b + 1) * cfg.W)
                        if out_f32:
                            nc.scalar.activation(outT[:, bsl], ps[:],
                                                 AF.Identity, bias=bias_col)
                        else:
                            nc.scalar.copy(outT[:, bsl], ps[:])

                def gcn_dense(Xt, hT, s2T, label):
                    for c in range(cfg.NCHUNK):
                        csl = slice(c * cfg.CH, (c + 1) * cfg.CH)
                        for m in range(MG):
                            hp = app.tile([P, cfg.CH], F32,
                                          name=f"hp_{label}_{c}_{m}",
                                          tag="mm2", bufs=2)
                            nc.tensor.matmul(hp[:],
                                             gW1_sb[:, m * P:(m + 1) * P],
                                             Xt[:, csl], start=True,
                                             stop=True)
                            nc.scalar.activation(hT[m][:, csl], hp[:],
                                                 AF.Relu,
                                                 bias=gb1_sb[:, m:m + 1])
                        sp = app.tile([P, cfg.CH], F32, name=f"sp_{label}_{c}",
                                      tag="mm2", bufs=2)
                        for k in range(MG):
                            nc.tensor.matmul(sp[:],
                                             gW2_sb[:, k * P:(k + 1) * P],
                                             hT[k][:, csl],
                                             start=(k == 0),
                                             stop=(k == MG - 1))
                        nc.scalar.copy(s2T[:, csl], sp[:])

                padsl = (slice(cfg.NREAL * cfg.W, cfg.RPAD)
                         if cfg.NREAL * cfg.W < cfg.RPAD else None)

                # graph f layer 1
                Xt_f = bigp.tile([P, cfg.RPAD], BF16, name="Xt_f", **BIGT)
                if padsl:
                    nc.vector.memset(Xt_f[:, padsl], 0.0)
                aggregate(z_full, gstreams["f"], Xt_f, None, False, "f1")
                hT_f = [bigp.tile([P, cfg.RPAD], BF16, name=f"hT_f{m}",
                                  **BIGT) for m in range(MG)]
                s2T_f = bigp.tile([P, cfg.RPAD], BF16, name="s2T_f", **BIGT)
                if debug_taps:
                    nc.sync.dma_start(xf_dbg[:, :], Xt_f[:])
                gcn_dense(Xt_f, hT_f, s2T_f, "f")
                store_rows(s2T_f, s2f_loc)
                nc.gpsimd.collective_compute(
                    "AllGather", ALU.bypass, replica_groups=rg,
                    ins=[s2f_loc.opt()], outs=[s2f_full.opt()])
                if debug_taps:
                    nc.sync.dma_start(s2f_dbg[:, :], s2f_full[:, :])

                # graph s layer 1 (overlaps the s2f all-gather)
                Xt_s = bigp.tile([P, cfg.RPAD], BF16, name="Xt_s", **BIGT)
                if padsl:
                    nc.vector.memset(Xt_s[:, padsl], 0.0)
                aggregate(z_full, gstreams["s"], Xt_s, None, False, "s1")
                hT_s = [bigp.tile([P, cfg.RPAD], BF16, name=f"hT_s{m}",
                                  **BIGT) for m in range(MG)]
                s2T_s = bigp.tile([P, cfg.RPAD], BF16, name="s2T_s", **BIGT)
                gcn_dense(Xt_s, hT_s, s2T_s, "s")
                store_rows(s2T_s, s2s_loc)
                nc.gpsimd.collective_compute(
                    "AllGather", ALU.bypass, replica_groups=rg,
                    ins=[s2s_loc.opt()], outs=[s2s_full.opt()])

                # layer 2 aggregations
                femT = bigf.tile([P, cfg.RPAD], F32, name="femT",
                                 tag="bigf32", bufs=2)
                if padsl:
                    nc.vector.memset(femT[:, padsl], 0.0)
                aggregate(s2f_full, gstreams["f"], femT, gb2_sb[:, :1], True,
                          "f2")
                if debug_taps:
                    nc.sync.dma_start(fem_dbg[:, :], femT[:])
                semT = bigf.tile([P, cfg.RPAD], F32, name="semT",
                                 tag="bigf32", bufs=2)
                if padsl:
                    nc.vector.memset(semT[:, padsl], 0.0)
                aggregate(s2s_full, gstreams["s"], semT, gb2_sb[:, :1], True,
                          "s2")

                # ============= attention + emb =============
                embT = bigp.tile([P, cfg.RPAD], BF16, name="embT", **BIGT)
                for rt in range(cfg.RT):
                    rsl = slice(rt * P, (rt + 1) * P)
                    fp_ = ptr.tile([P, P], F32, name=f"fp_{rt}", tag="tr",
                                   bufs=2)
                    nc.tensor.transpose(fp_[:], femT[:, rsl], id32[:])
                    frow = st.tile([P, P], F32, name=f"frow_{rt}", tag="frow",
                                   bufs=2)
                    nc.scalar.copy(frow[:], fp_[:])
                    nc.sync.dma_start(femb_o[rsl, :], frow[:])
                    sp_ = ptr.tile([P, P], F32, name=f"spt_{rt}", tag="tr",
                                   bufs=2)
                    nc.tensor.transpose(sp_[:], semT[:, rsl], id32[:])
                    srow = st.tile([P, P], F32, name=f"srow_{rt}", tag="srow",
                                   bufs=2)
                    nc.scalar.copy(srow[:], sp_[:])
                    nc.sync.dma_start(semb_o[rsl, :], srow[:])

                    tmpf = st.tile([P, P], F32, name=f"tmpf_{rt}", tag="atmp",
                                   bufs=2)
                    nc.vector.tensor_tensor(tmpf[:], frow[:], aWb_sb[:],
                                            op=ALU.mult)
                    wf = st.tile([P, 1], F32, name=f"wf_{rt}", tag="wf",
                                 bufs=2)
                    nc.vector.tensor_reduce(wf[:], tmpf[:],
                                            axis=mybir.AxisListType.X,
                                            op=ALU.add)
                    tmps = st.tile([P, P], F32, name=f"tmps_{rt}",
                                   tag="atmp2", bufs=2)
                    nc.vector.tensor_tensor(tmps[:], srow[:], aWb_sb[:],
                                            op=ALU.mult)
                    ws = st.tile([P, 1], F32, name=f"ws_{rt}", tag="ws",
                                 bufs=2)
                    nc.vector.tensor_reduce(ws[:], tmps[:],
                                            axis=mybir.AxisListType.X,
                                            op=ALU.add)
                    dl = st.tile([P, 1], F32, name=f"dl_{rt}", tag="dl",
                                 bufs=2)
                    nc.vector.tensor_tensor(dl[:], wf[:], ws[:],
                                            op=ALU.subtract)
                    bq = st.tile([P, 1], F32, name=f"bq_{rt}", tag="bq",
                                 bufs=2)
                    nc.scalar.activation(bq[:], dl[:], AF.Sigmoid)
                    dif = st.tile([P, P], F32, name=f"dif_{rt}", tag="dif",
                                  bufs=2)
                    nc.vector.tensor_tensor(dif[:], frow[:], srow[:],
                                            op=ALU.subtract)
                    scd = st.tile([P, P], F32, name=f"scd_{rt}", tag="scd",
                                  bufs=2)
                    nc.vector.tensor_scalar(scd[:], dif[:], bq[:, :1], None,
                                            op0=ALU.mult)
                    er = st.tile([P, P], F32, name=f"er_{rt}", tag="er",
                                 bufs=2)
                    nc.vector.tensor_tensor(er[:], srow[:], scd[:],
                                            op=ALU.add)
                    nc.sync.dma_start(emb_o[rsl, :], er[:])
                    ep_ = ptr.tile([P, P], F32, name=f"ept_{rt}", tag="tr",
                                   bufs=2)
                    nc.tensor.transpose(ep_[:], er[:], id32[:])
                    nc.scalar.copy(embT[:, rsl], ep_[:])

            # ================= decoder =================
            with (
                tc.tile_pool(name="decw", bufs=1) as dw,
                tc.tile_pool(name="decs", bufs=1) as st,
                tc.tile_pool(name="decp", bufs=1, space="PSUM") as dpp,
            ):
                dW1_sb = []
                for k in range(MD):
                    t = dw.tile([P, cfg.DOUT], BF16, name=f"dW1_{k}",
                                tag=f"dw1_{k}")
                    nc.sync.dma_start(t[:], dW1p[k * P:(k + 1) * P, :])
                    dW1_sb.append(t)
                dbt1_sb = dw.tile([P, cfg.DOUT], F32, name="dbt1_sb")
                nc.sync.dma_start(dbt1_sb[:], dbt1b[:, :])

                dT = [bigp.tile([P, cfg.RPAD], BF16, name=f"dT_{m}", **BIGT)
                      for m in range(MD)]
                for c in range(cfg.NCHUNK):
                    csl = slice(c * cfg.CH, (c + 1) * cfg.CH)
                    for m in range(MD):
                        dps = dpp.tile([P, cfg.CH], F32, name=f"dps_{c}_{m}",
                                       tag="mmd", bufs=3)
                        nc.tensor.matmul(dps[:], dW0_sb[:, m * P:(m + 1) * P],
                                         embT[:, csl], start=True, stop=True)
                        _elu(nc, st, dps[:], dbt0_sb[:, m:m + 1],
                             dbt0n_sb[:, m:m + 1], dT[m][:, csl],
                             f"d_{c}_{m}")

                nsl = []
                n0 = 0
                while n0 < cfg.DOUT:
                    nsz = min(512, cfg.DOUT - n0)
                    nsl.append((n0, nsz))
                    n0 += nsz
                for rt in range(cfg.RT):
                    rsl = slice(rt * P, (rt + 1) * P)
                    for (n0, nsz) in nsl:
                        dep = dpp.tile([P, nsz], F32, name=f"dep_{rt}_{n0}",
                                       tag="mmd", bufs=3)
                        for k in range(MD):
                            nc.tensor.matmul(dep[:], dT[k][:, rsl],
                                             dW1_sb[k][:, n0:n0 + nsz],
                                             start=(k == 0),
                                             stop=(k == MD - 1))
                        det = st.tile([P, nsz], F32, name=f"det_{rt}_{n0}",
                                      tag="det", bufs=2)
                        nc.vector.tensor_tensor(det[:], dep[:],
                                                dbt1_sb[:, n0:n0 + nsz],
                                                op=ALU.add)
                        des = st.tile([P, nsz], F32, name=f"des_{rt}_{n0}",
                                      tag="des", bufs=2)
                        nc.scalar.activation(des[:], det[:], AF.Sigmoid)
                        nc.sync.dma_start(de_o[rsl, n0:n0 + nsz], des[:])

    nc.compile()
    return nc


# --------------------------------------------------------------------------
# entry point
# --------------------------------------------------------------------------

def run(inputs, cfg, balance=True, trace=False, sim=False, debug_taps=False,
        tmpdir=None):
    in_maps, meta = prep_host(inputs, cfg, balance=balance)
    nc = build_program(cfg, meta['str_f'], meta['str_s'],
                       debug_taps=debug_taps)
    if sim:
        from concourse.bass_interp import MultiCoreSim
        msim = MultiCoreSim(nc, num_cores=NCORES, trace=False,
                            require_finite=False, require_nnan=False)
        for c in range(NCORES):
            for k, v in in_maps[c].items():
                msim.cores[c].tensor(k)[:] = v
        msim.simulate(check_with_hw=False)
        results = []
        for c in range(NCORES):
            results.append({n: np.array(msim.cores[c].tensor(n))
                            for n in ('femb_o', 'semb_o', 'emb_o', 'de_o')})
        return assemble(results, cfg, meta), None
    res = run_bass_kernel_spmd(nc, in_maps, list(range(NCORES)), trace=trace,
                               tmpdir=tmpdir)
    out = assemble(res.results, cfg, meta)
    return out, res


def assemble(results, cfg, meta):
    N, R, EMB, DOUT = cfg.N, cfg.R, cfg.EMB, cfg.DOUT
    femb = np.empty((N, EMB), np.float32)
    semb = np.empty((N, EMB), np.float32)
    emb = np.empty((N, EMB), np.float32)
    de = np.empty((N, DOUT), np.float32)
    for c in range(NCORES):
        perm = meta['perms'][c]          # node i -> padded row perm[i]
        sl = slice(c * R, (c + 1) * R)
        femb[sl] = results[c]['femb_o'][perm]
        semb[sl] = results[c]['semb_o'][perm]
        emb[sl] = results[c]['emb_o'][perm]
        de[sl] = results[c]['de_o'][perm]
    return femb, semb, de, emb


def kernel(**inputs):
    out, _ = run(inputs, FULL, balance=True, trace=False)
    return out
